# revision 58
# baseline (speedup 1.0000x reference)
"""Causal self-attention (B=2, T=2048, C=1024, nh=16) on 8 TRN2 NeuronCores.

Sharding: core c = 4*b + g handles batch b (2048 tokens) and head-group g
(4 heads).  Megatron-style: QKV rows and proj columns sharded by head group;
the proj partial sums are reduced on the host (the "all-reduce").

Per-core kernel:
  1. x and W_kqv ship as bf16 (halves input DMA); QKV projection
     kqvT[f,t] = Wl @ x_b.T per 512-token chunk on the PE at full rate.
     Wl rows are pre-ordered [kT01|q01|v01|kT23|q23|v23] so weights arrive
     in consumption order and head-pair 01's attention starts as soon as
     the first third has landed.  kT / q for a head pair land merged (even
     head on partitions 0:64, odd on 64:128) in bf16; v stays f32r for the
     PE transposes.
  2. QK matmuls are K=64 bf16 and run pairwise-concurrent on the PE via
     tile_position row tiling (even head rows 0:64, odd rows 64:128).
  3. v tiles transposed on PE to [s,d], packed next to all-ones column
     blocks (memset on device) so the PV matmul also emits softmax row-sums
     broadcast across the complement 64 partitions.
  4. Software-pipelined schedule: attention j-steps of chunk n interleave
     with QKV matmul groups of chunk n+1 and output-projection groups of
     chunk n-1; garbage-input warmup matmuls spin the PE clock (HAM) while
     the first DMAs land.
  5. Engine balance: ACT does exp (+ last-chunk proj copies); DVE does all
     other PSUM reads (bias-adds, rowsum copies, y copies, mid-chunk proj
     copies) and the fast-approx reciprocals (full 128 partitions only --
     the custom DVE op misbehaves at base partition 64); GPSIMD does
     causal-diag masks and the normalization multiplies.  The rowsum
     partition-shift DMAs ride the ACT hwdge queue to dodge the output-DMA
     backlog on the sync queue.
  6. Partial projections stream out as bf16; the host cross-head-group
     reduction accumulates in f32.
"""

import os
import numpy as np

B, T, C, NH, HD = 2, 2048, 1024, 16, 64
HPC = 4  # heads per core
NCORES = 8
NCH = 4       # 512-wide t-chunks
CHW = 512

_cache = {}


def _build_nc():
    from contextlib import ExitStack

    import concourse.bass as bass
    import concourse.tile as tile
    from concourse import bacc, mybir

    f32 = mybir.dt.float32
    f32r = mybir.dt.float32r
    bf16 = mybir.dt.bfloat16
    AF = mybir.ActivationFunctionType
    OP = mybir.AluOpType

    nc = bacc.Bacc("TRN2", target_bir_lowering=False, debug=False,
                   num_devices=NCORES)

    xt = nc.dram_tensor("xt", [C, T], bf16, kind="ExternalInput").ap()
    wkqv = nc.dram_tensor("wkqv", [C, 3 * HPC * HD], bf16,
                          kind="ExternalInput").ap()
    bkq = nc.dram_tensor("bkq", [128, 6], f32, kind="ExternalInput").ap()
    wproj = nc.dram_tensor("wproj", [HPC * HD, C], bf16,
                           kind="ExternalInput").ap()
    bp = nc.dram_tensor("bp", [128, 8], f32, kind="ExternalInput").ap()
    ident_d = nc.dram_tensor("ident", [128, 128], f32,
                             kind="ExternalInput").ap()
    amask_d = nc.dram_tensor("amask", [128, 256], f32r,
                             kind="ExternalInput").ap()
    outp = nc.dram_tensor("outp", [C, T], bf16, kind="ExternalOutput").ap()
    dbgo = None
    if os.environ.get("KERNEL_DEBUG_DUMP"):
        dbgo = nc.dram_tensor("dbgo", [768, CHW], f32,
                              kind="ExternalOutput").ap()

    with tile.TileContext(nc) as tc, ExitStack() as ctx:
        sing = ctx.enter_context(tc.tile_pool(name="sing", bufs=1))
        xpool = ctx.enter_context(tc.tile_pool(name="xpool", bufs=2))
        ptp = ctx.enter_context(tc.tile_pool(name="ptp", bufs=4))
        rsp = ctx.enter_context(tc.tile_pool(name="rsp", bufs=2))
        rbp = ctx.enter_context(tc.tile_pool(name="rbp", bufs=2))
        osp = ctx.enter_context(tc.tile_pool(name="osp", bufs=6))
        ps = ctx.enter_context(tc.tile_pool(name="ps", bufs=2, space="PSUM"))

        # ---- resident SBUF tensors ----
        wk = sing.tile([128, 8, 768], bf16, name="wk")
        # q by head-pair: even head partitions 0:64, odd 64:128
        qsb = sing.tile([128, 2, T], bf16, name="qsb")
        # kT by head-pair, same partition split
        ktp = sing.tile([128, 2, T], bf16, name="ktp")
        # vT by head-pair (f32r: feeds PE transposes)
        vv = sing.tile([128, 2, T], f32r, name="vv")
        # vsb: 32 blocks of [v_A(64) | ones(128) | v_B(64)]
        vsb = sing.tile([128, 32 * 256], f32r, name="vsb")
        ysb = sing.tile([128, 2, T], bf16, name="ysb")
        wp = sing.tile([128, 2, C], bf16, name="wp")
        bkq_s = sing.tile([128, 6], f32, name="bkq_s")
        bp_s = sing.tile([128, 8], f32, name="bp_s")
        ident = sing.tile([128, 128], f32, name="ident")
        amask = sing.tile([128, 2, 128], f32r, name="amask")
        warm = sing.tile([128, CHW], bf16, name="warm")

        xt_r = xt.rearrange("(kk p) t -> p kk t", p=128)
        wkqv_r = wkqv.rearrange("(kk p) f -> p kk f", p=128)

        # all-ones mid columns of vsb, generated on device
        vanchor = vsb[:, 64:65]
        ones_view = bass.AP(tensor=vanchor.tensor, offset=vanchor.offset,
                            ap=[vanchor.ap[0], [256, 32], [1, 128]])
        nc.gpsimd.memset(ones_view.bitcast(f32), 1.0)

        # ---- initial DMAs, ordered by first consumption ----
        # wk column blocks (new order): 0:256 = kT01|q01, 256:512 = v01|kT23,
        # 512:768 = q23|v23
        xts0 = xpool.tile([128, 8, CHW], bf16, name="xts")
        for k in range(8):
            nc.sync.dma_start(xts0[:, k, :], xt_r[:, k, 0:CHW])
            nc.scalar.dma_start(wk[:, k, 0:256], wkqv_r[:, k, 0:256])
        nc.sync.dma_start(bkq_s, bkq)
        nc.sync.dma_start(ident, ident_d)
        nc.sync.dma_start(amask, amask_d.rearrange("p (a b) -> p a b", a=2))
        for k in range(8):
            (nc.sync if k % 2 else nc.scalar).dma_start(
                wk[:, k, 256:512], wkqv_r[:, k, 256:512])
        for k in range(8):
            (nc.sync if k % 2 else nc.scalar).dma_start(
                wk[:, k, 512:768], wkqv_r[:, k, 512:768])
        wp_r = wproj.rearrange("(kk p) f -> p kk f", p=128)
        nc.sync.dma_start(wp[:, 0, :], wp_r[:, 0, :])
        nc.scalar.dma_start(wp[:, 1, :], wp_r[:, 1, :])
        nc.sync.dma_start(bp_s, bp)

        xts_tiles = {0: xts0}

        def v_stationary(j, h):
            """[128,128] AP: even slot -> [v_A|ones64], odd -> [ones64|v_B]."""
            hf2, sl = h // 2, h % 2
            off = (j * 2 + hf2) * 256 + 128 * sl
            return vsb[:, off:off + 128]

        # ---- step emitters ----
        # m order: 0=kT01, 1=q01, 2=v01, 3=kT23, 4=q23, 5=v23
        M_DST = [(0, 0), (1, 0), (2, 0), (0, 1), (1, 1), (2, 1)]

        def qkv_m_step(n, m):
            with nc.named_scope(f"qkv{n}"):
                xts = xts_tiles[n]
                cols = slice(n * CHW, (n + 1) * CHW)
                acc = ps.tile([128, CHW], f32, name="acc", tag="acc")
                for k in range(8):
                    nc.tensor.matmul(acc, wk[:, k, m * 128:(m + 1) * 128],
                                     xts[:, k, :], start=(k == 0),
                                     stop=(k == 7))
                kind, hf = M_DST[m]
                dst = (ktp, qsb, vv)[kind]
                nc.vector.tensor_scalar_add(
                    out=dst[:, hf, cols], in0=acc, scalar1=bkq_s[:, m:m + 1])
                if m == 0 and n + 1 < NCH:
                    nxt = xpool.tile([128, 8, CHW], bf16, name="xts")
                    for k in range(8):
                        (nc.sync if k % 2 else nc.scalar).dma_start(
                            nxt[:, k, :],
                            xt_r[:, k, (n + 1) * CHW:(n + 2) * CHW])
                    xts_tiles[n + 1] = nxt

        def transpose_step(n, t2, hf):
            with nc.named_scope(f"attn{n}"):
                jb = 4 * n + 2 * t2
                tp = ps.tile([128, 2, 128], f32, name="tp", tag="acc")
                for ji in range(2):
                    nc.tensor.transpose(
                        tp[:, ji, :],
                        vv[:, hf,
                           (jb + ji) * 128:(jb + ji + 1) * 128].bitcast(f32),
                        ident)
                off = jb * 512 + hf * 256
                anch = vsb[:, off:off + 1]
                dst = bass.AP(tensor=anch.tensor, offset=anch.offset,
                              ap=[anch.ap[0], [512, 2], [192, 2], [1, 64]])
                nc.vector.tensor_copy(
                    dst, tp.rearrange("p j (a b) -> p j a b", a=2))

        pend = []

        def flush_pend(k=None):
            todo = len(pend) if k is None else k
            for _ in range(todo):
                pend.pop(0)()

        cur_pys = [None, None]

        def hf_start_step(n, hf):
            cur_pys[0] = ps.tile([128, CHW], f32, name="py0", tag="py")
            cur_pys[1] = ps.tile([128, CHW], f32, name="py1", tag="py")

        defer3 = []

        def qk_step(n, hf, j, defer=False):
            with nc.named_scope(f"attn{n}"):
                c0 = max(0, 128 * j - CHW * n)
                ss = ps.tile([128, 2, CHW], f32, name="ss", tag="ss")
                for sl in range(2):
                    nc.tensor.matmul(
                        ss[:, sl, c0:],
                        ktp[64 * sl:64 * sl + 64, hf, j * 128:(j + 1) * 128],
                        qsb[64 * sl:64 * sl + 64, hf,
                            n * CHW + c0:(n + 1) * CHW],
                        start=True, stop=True,
                        tile_position=(64 * sl, 0))
                if defer:
                    pt = ptp.tile([128, 2, CHW], f32r, name="pt3",
                                  tag="pt3", bufs=8)
                else:
                    pt = ptp.tile([128, 2, CHW], f32r, name="pt")
                nc.scalar.activation(out=pt[:, :, c0:], in_=ss[:, :, c0:],
                                     func=AF.Exp)
                if os.environ.get("KERNEL_DEBUG_DUMP") and (n, hf, j) == (1, 0, 0):
                    dbg3 = sing.tile([128, 2, CHW], f32, name="dbg3")
                    nc.vector.tensor_copy(dbg3[:, 0, :], ss[:, 0, :])
                    nc.vector.tensor_copy(dbg3[:, 1, :], pt[:, 0, :].bitcast(f32))
                    nc.sync.dma_start(dbgo[0:128, :], dbg3[:, 0, :])
                    nc.sync.dma_start(dbgo[128:256, :], dbg3[:, 1, :])
                if j >= 4 * n:  # diagonal block: 0/1 mask post-exp
                    nc.gpsimd.tensor_tensor(
                        out=pt[:, :, c0:c0 + 128],
                        in0=pt[:, :, c0:c0 + 128],
                        in1=amask, op=OP.mult)
                jmax = 4 * n + 3
                # deferred steps resolve the py accumulators at call time
                # (their hf_start runs later than this emission)
                pys = None if defer else tuple(cur_pys)

                def mk_pv(n=n, hf=hf, j=j, c0=c0, pt=pt, pys=pys, jmax=jmax):
                    py01 = tuple(cur_pys) if pys is None else pys
                    with nc.named_scope(f"attn{n}"):
                        for sl in range(2):
                            nc.tensor.matmul(
                                py01[sl][:, c0:],
                                v_stationary(j, 2 * hf + sl),
                                pt[:, sl, c0:],
                                start=(j == 0), stop=(j == jmax),
                                skip_group_check=True)
                if defer:
                    defer3.append(mk_pv)
                else:
                    pend.append(mk_pv)
                    if len(pend) > 2:
                        flush_pend(1)

        def norm_step(n, hf):
            with nc.named_scope(f"attn{n}"):
                flush_pend()
                pys = cur_pys
                # y of head A at 0:64 (pys0) / head B at 64:128 (pys1);
                # rowsum broadcasts live on the complements.  Copy rowsums
                # to SBUF, one full-partition fast reciprocal, DMA
                # partition-shift on the ACT queue, one gpsimd multiply.
                # The last norm routes its copies to the then-idle ACT so
                # the DVE chain shortens at the tail.
                tail = (n, hf) == (3, 1)

                def cpy(dst, src):
                    if tail:
                        nc.scalar.activation(out=dst, in_=src,
                                             func=AF.Identity, bias=0.0)
                    else:
                        nc.vector.tensor_copy(dst, src)

                pyc = rsp.tile([128, CHW], f32, name="pyc", tag="pyc")
                rcp = rsp.tile([128, CHW], f32, name="rcp", tag="rcp")
                rb = rbp.tile([128, CHW], f32, name="rb")
                if tail:
                    # column halves, sum-copies split ACT/DVE in parallel,
                    # so the first half of the reciprocals lands sooner
                    for ch in range(2):
                        s = slice(ch * 256, ch * 256 + 256)
                        nc.scalar.activation(out=pyc[0:64, s],
                                             in_=pys[1][0:64, s],
                                             func=AF.Identity, bias=0.0)
                        nc.vector.tensor_copy(pyc[64:128, s],
                                              pys[0][64:128, s])
                        nc.vector.reciprocal_approx_fast(out=rcp[:, s],
                                                         in_=pyc[:, s])
                        nc.scalar.dma_start(rb[0:64, s], rcp[64:128, s])
                        nc.scalar.dma_start(rb[64:128, s], rcp[0:64, s])
                else:
                    cpy(pyc[0:64, :], pys[1][0:64, :])
                    cpy(pyc[64:128, :], pys[0][64:128, :])
                    nc.vector.reciprocal_approx_fast(out=rcp, in_=pyc)
                    nc.scalar.dma_start(rb[0:64, :], rcp[64:128, :])
                    nc.scalar.dma_start(rb[64:128, :], rcp[0:64, :])
                pyy = rsp.tile([128, CHW], f32, name="pyy", tag="pyy")
                if tail:  # halves: first 256 cols feed proj3 kk=1 sooner
                    for ch in range(2):
                        cs = slice(ch * 256, ch * 256 + 256)
                        nc.vector.tensor_copy(pyy[0:64, cs],
                                              pys[0][0:64, cs])
                        nc.scalar.activation(out=pyy[64:128, cs],
                                             in_=pys[1][64:128, cs],
                                             func=AF.Identity, bias=0.0)
                        nc.gpsimd.tensor_tensor(
                            out=ysb[:, hf, n * CHW + ch * 256:
                                    n * CHW + ch * 256 + 256],
                            in0=pyy[:, cs], in1=rb[:, cs], op=OP.mult)
                else:
                    nc.vector.tensor_copy(pyy[0:64, :], pys[0][0:64, :])
                    nc.vector.tensor_copy(pyy[64:128, :], pys[1][64:128, :])
                    nc.gpsimd.tensor_tensor(
                        out=ysb[:, hf, n * CHW:(n + 1) * CHW],
                        in0=pyy, in1=rb, op=OP.mult)
                if os.environ.get("KERNEL_DEBUG_DUMP") and (n, hf) == (1, 0):
                    dbg2 = sing.tile([128, 4, CHW], f32, name="dbg2")
                    nc.vector.tensor_copy(dbg2[:, 0, :], pyy)
                    nc.vector.tensor_copy(dbg2[:, 1, :], pyc)
                    nc.vector.tensor_copy(dbg2[:, 2, :], rb)
                    nc.vector.tensor_copy(dbg2[:, 3, :], pys[0])
                    for q4 in range(4):
                        nc.sync.dma_start(
                            dbgo[256 + q4 * 128:256 + (q4 + 1) * 128, :],
                            dbg2[:, q4, :])

        p3_accs = []

        def p3a_step():
            # After chunk 3's last QK: flush the remaining PVs, then park
            # kk=0 partial projections for o=0..5 in the now-free ss/acc
            # PSUM slots.  These matmuls keep the PE warm while the final
            # normalization chain runs on ACT/DVE/DMA.
            flush_pend()
            with nc.named_scope("proj3"):
                ssA = ps.tile([128, 2, CHW], f32, name="p3sA", tag="ss")
                ssB = ps.tile([128, 2, CHW], f32, name="p3sB", tag="ss")
                aA = ps.tile([128, CHW], f32, name="p3aA", tag="acc")
                aB = ps.tile([128, CHW], f32, name="p3aB", tag="acc")
                p3_accs.extend([ssA[:, 0, :], ssA[:, 1, :],
                                ssB[:, 0, :], ssB[:, 1, :], aA, aB])
                for o in range(6):
                    nc.tensor.matmul(
                        p3_accs[o], wp[:, 0, o * 128:(o + 1) * 128],
                        ysb[:, 0, 3 * CHW:4 * CHW],
                        start=True, stop=False, skip_group_check=True)

        def p3b_epilogue():
            with nc.named_scope("proj3"):
                for o in range(6, 8):
                    acc = ps.tile([128, CHW], f32, name="p3p", tag="py")
                    p3_accs.append(acc)
                    nc.tensor.matmul(
                        acc, wp[:, 0, o * 128:(o + 1) * 128],
                        ysb[:, 0, 3 * CHW:4 * CHW],
                        start=True, stop=False, skip_group_check=True)
                # kk=1 in column halves; once kk0(full) + kk1-h0 land, the
                # first 256 columns of an accumulator are final -- copy and
                # DMA them out while the second half still computes
                for hh in range(2):
                    cs = slice(3 * CHW + hh * 256, 3 * CHW + hh * 256 + 256)
                    for o in range(8):
                        nc.tensor.matmul(
                            p3_accs[o][:, hh * 256:hh * 256 + 256],
                            wp[:, 1, o * 128:(o + 1) * 128],
                            ysb[:, 1, cs],
                            start=False, stop=(hh == 1),
                            skip_group_check=True)
                        ot = osp.tile([128, 256], bf16, name="ot")
                        src = p3_accs[o][:, hh * 256:hh * 256 + 256]
                        if o % 2 == 0:
                            nc.scalar.activation(out=ot, in_=src,
                                                 func=AF.Identity,
                                                 bias=bp_s[:, o:o + 1])
                        else:
                            nc.vector.tensor_scalar_add(
                                out=ot, in0=src, scalar1=bp_s[:, o:o + 1])
                        eng = nc.sync if o % 2 == 0 else nc.scalar
                        eng.dma_start(
                            outp[o * 128:(o + 1) * 128, cs], ot)

        def proj_step(n, o):
            with nc.named_scope(f"proj{n}"):
                tag = "acc" if (n < 3 or o % 2 == 0) else "ss"
                acc = ps.tile([128, CHW], f32, name="pacc", tag=tag)
                for kk in range(2):
                    nc.tensor.matmul(acc, wp[:, kk, o * 128:(o + 1) * 128],
                                     ysb[:, kk, n * CHW:(n + 1) * CHW],
                                     start=(kk == 0), stop=(kk == 1))
                ot = osp.tile([128, CHW], bf16, name="ot")
                if n < 3:
                    nc.vector.tensor_scalar_add(out=ot, in0=acc,
                                                scalar1=bp_s[:, o:o + 1])
                else:
                    nc.scalar.activation(out=ot, in_=acc, func=AF.Identity,
                                         bias=bp_s[:, o:o + 1])
                nc.sync.dma_start(
                    outp[o * 128:(o + 1) * 128,
                         n * CHW:n * CHW + 256], ot[:, 0:256])
                nc.scalar.dma_start(
                    outp[o * 128:(o + 1) * 128,
                         n * CHW + 256:(n + 1) * CHW], ot[:, 256:512])

        def dispatch(step):
            kind = step[0]
            if kind == "qk":
                qk_step(*step[1:])
            elif kind == "qkvm":
                qkv_m_step(*step[1:])
            elif kind == "tr":
                transpose_step(*step[1:])
            elif kind == "hfs":
                hf_start_step(*step[1:])
            elif kind == "norm":
                norm_step(*step[1:])
            elif kind == "proj":
                proj_step(*step[1:])
            elif kind == "p3a":
                p3a_step()
            elif kind == "qk3e":
                qk_step(3, 0, step[1], defer=True)
            elif kind == "fl3":
                while defer3:
                    defer3.pop(0)()
            else:
                raise AssertionError(kind)

        # ---- prologue: PE warmup on garbage inputs + first head-pair's
        # QKV (m0 = kT01, m1 = q01) ----
        nc.vector.memset(warm, 1.0)
        # preload the exp table set on ACT while the first DMAs land
        wexp = sing.tile([128, 8], f32, name="wexp")
        nc.scalar.activation(out=wexp, in_=warm[:, 0:8], func=AF.Exp)
        for w in range(14):
            wps = ps.tile([128, CHW], f32, name="wps", tag="acc")
            nc.tensor.matmul(wps, warm[:, 0:128], warm, start=True, stop=True)
        qkv_m_step(0, 0)
        qkv_m_step(0, 1)

        # chunk 0 schedule, hand-ordered (v01 -> transposes(hf0) before the
        # first PV flush; kT23/q23 before hf1's QKs)
        sched = [("hfs", 0, 0), ("qk", 0, 0, 0), ("qkvm", 0, 2),
                 ("qk", 0, 0, 1), ("tr", 0, 0, 0), ("tr", 0, 1, 0),
                 ("qk", 0, 0, 2), ("qkvm", 0, 3), ("qk", 0, 0, 3),
                 ("qkvm", 0, 4), ("norm", 0, 0),
                 ("hfs", 0, 1), ("qk", 0, 1, 0), ("qkvm", 0, 5),
                 ("qk", 0, 1, 1), ("tr", 0, 0, 1), ("tr", 0, 1, 1),
                 ("qk", 0, 1, 2), ("qkvm", 1, 0), ("qk", 0, 1, 3),
                 ("qkvm", 1, 1), ("norm", 0, 1),
                 ("qkvm", 1, 2), ("qkvm", 1, 3), ("qkvm", 1, 4),
                 ("qkvm", 1, 5)]
        for s in sched:
            dispatch(s)

        # chunks 1..3: attention interleaved with next chunk's QKV and
        # previous chunk's projection
        for n in range(1, NCH):
            attn = []
            for hf in range(2):
                attn.append(("hfs", n, hf))
                if n == 3 and hf == 0:
                    # j0..7's QK+exp ran during chunk 2; their PVs flush here
                    attn.append(("fl3",))
                j0 = 8 if (n == 3 and hf == 0) else 0
                for j in range(j0, 4 * n + 4):
                    attn.append(("qk", n, hf, j))
                attn.append(("norm", n, hf))
            # v transposes for this chunk, per head-pair, early in each hf
            anchor1 = ("qk", n, 0, 9) if n == 3 else ("qk", n, 0, 1)
            base1 = attn.index(anchor1) + 1
            attn.insert(base1, ("tr", n, 0, 0))
            attn.insert(base1 + 1, ("tr", n, 1, 0))
            base2 = attn.index(("qk", n, 1, 1)) + 1
            attn.insert(base2, ("tr", n, 0, 1))
            attn.insert(base2 + 1, ("tr", n, 1, 1))
            if n == 3:
                attn.insert(attn.index(("norm", 3, 1)), ("p3a",))

            # qkvm fillers pace over the first ~2/3 of qk steps; proj
            # fillers start a few steps in (the previous chunk's last norm
            # chain is still draining) and pace over the rest.  Chunk 3 is
            # ACT(exp)-limited, so both proj(1) and proj(2) land there as
            # extra PE work.
            qkvm_f = ([("qkvm", n + 1, m) for m in range(6)]
                      if n + 1 < NCH else [])
            if n == 1:
                proj_f = [("proj", 0, o) for o in range(8)]
            elif n == 2:
                proj_f = [("proj", 1, o) for o in range(4)]
            else:
                proj_f = [("proj", 1, o) for o in range(4, 8)]
                proj_f += [("proj", 2, o) for o in range(8)]
            # chunk 2 also hosts chunk 3 / hf0's early QK+exp steps
            e3 = [("qk3e", j) for j in range(8)] if n == 2 else []
            nqk = sum(1 for s in attn if s[0] == "qk")
            span_a = max(1, (2 * nqk) // 3)
            qi = ai = pi = ei = 0
            for s in attn:
                dispatch(s)
                if s[0] == "qk":
                    qi += 1
                    want_a = min(len(qkvm_f), (qi * len(qkvm_f)) // span_a)
                    while ai < want_a:
                        dispatch(qkvm_f[ai])
                        ai += 1
                    # complete proj fillers by ~5/8 of the chunk so their
                    # output DMAs drain before the tail
                    span_p = max(1, (5 * nqk) // 8 - 3)
                    if qi > 3:
                        want_p = min(len(proj_f),
                                     ((qi - 3) * len(proj_f)) // span_p)
                        while pi < want_p:
                            dispatch(proj_f[pi])
                            pi += 1
                    if e3 and qi > 8:
                        want_e = min(len(e3),
                                     ((qi - 8) * len(e3)) // (nqk - 8))
                        while ei < want_e:
                            dispatch(e3[ei])
                            ei += 1
            while ai < len(qkvm_f):
                dispatch(qkvm_f[ai])
                ai += 1
            while pi < len(proj_f):
                dispatch(proj_f[pi])
                pi += 1
            while ei < len(e3):
                dispatch(e3[ei])
                ei += 1

        # epilogue: finish the last chunk's projection (kk=1 accumulate +
        # bias copies on ACT + output DMA)
        p3b_epilogue()

    nc.compile()
    return nc


def _host_inputs(x, W_kqv, b_kqv, W_proj, b_proj):
    import ml_dtypes

    bf16 = ml_dtypes.bfloat16
    x = np.ascontiguousarray(np.asarray(x, dtype=np.float32))
    W_kqv = np.asarray(W_kqv, dtype=np.float32)
    b_kqv = np.asarray(b_kqv, dtype=np.float32)
    W_proj = np.asarray(W_proj, dtype=np.float32)
    b_proj = np.asarray(b_proj, dtype=np.float32)

    ident = np.eye(128, dtype=np.float32)
    ss, tt = np.meshgrid(np.arange(128), np.arange(128), indexing="ij")
    amask = np.tile((ss <= tt).astype(np.float32), (1, 2))  # 0/1 mult mask

    xts = [np.ascontiguousarray(x[b].T.astype(bf16)) for b in range(B)]

    in_maps = []
    for c in range(NCORES):
        b, g = c // 4, c % 4
        heads = [4 * g + i for i in range(HPC)]

        def kqv_block(pair):
            hs = heads[2 * pair:2 * pair + 2]
            return [np.concatenate([W_kqv[h * 192:h * 192 + 64]
                                    for h in hs], axis=0),
                    np.concatenate([W_kqv[h * 192 + 64:h * 192 + 128] * 0.125
                                    for h in hs], axis=0),
                    np.concatenate([W_kqv[h * 192 + 128:h * 192 + 192]
                                    for h in hs], axis=0)]

        def bias_block(pair):
            hs = heads[2 * pair:2 * pair + 2]
            return [np.concatenate([b_kqv[h * 192:h * 192 + 64]
                                    for h in hs]),
                    np.concatenate([b_kqv[h * 192 + 64:h * 192 + 128] * 0.125
                                    for h in hs]),
                    np.concatenate([b_kqv[h * 192 + 128:h * 192 + 192]
                                    for h in hs])]

        # m order: kT01, q01, v01, kT23, q23, v23
        wl = np.concatenate(kqv_block(0) + kqv_block(1), axis=0)
        bl = np.concatenate(bias_block(0) + bias_block(1))
        bpl = b_proj if g == 0 else np.zeros_like(b_proj)
        in_maps.append({
            "xt": xts[b],
            "wkqv": np.ascontiguousarray(wl.T.astype(bf16)),
            "bkq": np.ascontiguousarray(bl.reshape(6, 128).T),
            "bp": np.ascontiguousarray(bpl.reshape(8, 128).T),
            "wproj": np.ascontiguousarray(
                W_proj[:, 256 * g:256 * (g + 1)].T.astype(bf16)),
            "ident": ident,
            "amask": amask,
        })
    return in_maps


def kernel(x, W_kqv, b_kqv, W_proj, b_proj):
    from concourse.bass_utils import run_bass_kernel_spmd

    if "nc" not in _cache:
        _cache["nc"] = _build_nc()
    nc = _cache["nc"]

    in_maps = _host_inputs(x, W_kqv, b_kqv, W_proj, b_proj)
    trace = bool(int(os.environ.get("KERNEL_TRACE", "0")))
    r = run_bass_kernel_spmd(nc, in_maps, core_ids=list(range(NCORES)),
                             trace=trace)
    if trace:
        _cache["last_results"] = r
        print(f"HW exec time: {r.exec_time_ns} ns")

    out = np.empty((B, T, C), dtype=np.float32)
    for b in range(B):
        acc = np.zeros((C, T), dtype=np.float32)
        for g in range(4):
            acc += r.results[4 * b + g]["outp"].astype(np.float32)
        out[b] = acc.T
    return out


# revision 60
# speedup vs baseline: 1.0387x; 1.0387x over previous
"""Causal self-attention (B=2, T=2048, C=1024, nh=16) on 8 TRN2 NeuronCores.

Sharding: core c = 4*b + g handles batch b (2048 tokens) and head-group g
(4 heads).  Megatron-style: QKV rows and proj columns sharded by head group;
the proj partial sums are reduced on the host (the "all-reduce").

Per-core kernel:
  1. x and W_kqv ship as bf16 (halves input DMA); QKV projection
     kqvT[f,t] = Wl @ x_b.T per 512-token chunk on the PE at full rate.
     Wl rows are pre-ordered [kT01|q01|v01|kT23|q23|v23] so weights arrive
     in consumption order and head-pair 01's attention starts as soon as
     the first third has landed.  kT / q for a head pair land merged (even
     head on partitions 0:64, odd on 64:128) in bf16; v stays f32r for the
     PE transposes.
  2. QK matmuls are K=64 bf16 and run pairwise-concurrent on the PE via
     tile_position row tiling (even head rows 0:64, odd rows 64:128).
  3. v tiles transposed on PE to [s,d], packed next to all-ones column
     blocks (memset on device) so the PV matmul also emits softmax row-sums
     broadcast across the complement 64 partitions.
  4. Software-pipelined schedule: attention j-steps of chunk n interleave
     with QKV matmul groups of chunk n+1 and output-projection groups of
     chunk n-1; garbage-input warmup matmuls spin the PE clock (HAM) while
     the first DMAs land.
  5. Engine balance: ACT does exp (+ last-chunk proj copies); DVE does all
     other PSUM reads (bias-adds, rowsum copies, y copies, mid-chunk proj
     copies) and the fast-approx reciprocals (full 128 partitions only --
     the custom DVE op misbehaves at base partition 64); GPSIMD does
     causal-diag masks and the normalization multiplies.  The rowsum
     partition-shift DMAs ride the ACT hwdge queue to dodge the output-DMA
     backlog on the sync queue.
  6. Partial projections stream out as bf16; the host cross-head-group
     reduction accumulates in f32.
"""

import os
import numpy as np

B, T, C, NH, HD = 2, 2048, 1024, 16, 64
HPC = 4  # heads per core
NCORES = 8
NCH = 4       # 512-wide t-chunks
CHW = 512

_cache = {}


def _build_nc():
    from contextlib import ExitStack

    import concourse.bass as bass
    import concourse.tile as tile
    from concourse import bacc, mybir

    f32 = mybir.dt.float32
    f32r = mybir.dt.float32r
    bf16 = mybir.dt.bfloat16
    AF = mybir.ActivationFunctionType
    OP = mybir.AluOpType

    nc = bacc.Bacc("TRN2", target_bir_lowering=False, debug=False,
                   num_devices=NCORES)

    xt = nc.dram_tensor("xt", [C, T], bf16, kind="ExternalInput").ap()
    wkqv = nc.dram_tensor("wkqv", [C, 3 * HPC * HD], bf16,
                          kind="ExternalInput").ap()
    bkq = nc.dram_tensor("bkq", [128, 6], f32, kind="ExternalInput").ap()
    wproj = nc.dram_tensor("wproj", [HPC * HD, C], bf16,
                           kind="ExternalInput").ap()
    bp = nc.dram_tensor("bp", [128, 8], f32, kind="ExternalInput").ap()
    ident_d = nc.dram_tensor("ident", [128, 128], f32,
                             kind="ExternalInput").ap()
    amask_d = nc.dram_tensor("amask", [128, 256], f32r,
                             kind="ExternalInput").ap()
    outp = nc.dram_tensor("outp", [C, T], bf16, kind="ExternalOutput").ap()
    dbgo = None
    if os.environ.get("KERNEL_DEBUG_DUMP"):
        dbgo = nc.dram_tensor("dbgo", [768, CHW], f32,
                              kind="ExternalOutput").ap()

    with tile.TileContext(nc) as tc, ExitStack() as ctx:
        sing = ctx.enter_context(tc.tile_pool(name="sing", bufs=1))
        xpool = ctx.enter_context(tc.tile_pool(name="xpool", bufs=2))
        ptp = ctx.enter_context(tc.tile_pool(name="ptp", bufs=4))
        rsp = ctx.enter_context(tc.tile_pool(name="rsp", bufs=2))
        rbp = ctx.enter_context(tc.tile_pool(name="rbp", bufs=2))
        osp = ctx.enter_context(tc.tile_pool(name="osp", bufs=6))
        ps = ctx.enter_context(tc.tile_pool(name="ps", bufs=2, space="PSUM"))

        # ---- resident SBUF tensors ----
        wk = sing.tile([128, 8, 768], bf16, name="wk")
        # q by head-pair: even head partitions 0:64, odd 64:128
        qsb = sing.tile([128, 2, T], bf16, name="qsb")
        # kT by head-pair, same partition split
        ktp = sing.tile([128, 2, T], bf16, name="ktp")
        # vT by head-pair (f32r: feeds PE transposes)
        vv = sing.tile([128, 2, T], f32r, name="vv")
        # vsb: 32 blocks of [v_A(64) | ones(128) | v_B(64)]
        vsb = sing.tile([128, 32 * 256], f32r, name="vsb")
        ysb = sing.tile([128, 2, T], bf16, name="ysb")
        wp = sing.tile([128, 2, C], bf16, name="wp")
        bkq_s = sing.tile([128, 6], f32, name="bkq_s")
        bp_s = sing.tile([128, 8], f32, name="bp_s")
        ident = sing.tile([128, 128], f32, name="ident")
        amask = sing.tile([128, 2, 128], f32r, name="amask")
        warm = sing.tile([128, CHW], bf16, name="warm")

        xt_r = xt.rearrange("(kk p) t -> p kk t", p=128)
        wkqv_r = wkqv.rearrange("(kk p) f -> p kk f", p=128)

        # all-ones mid columns of vsb, generated on device
        vanchor = vsb[:, 64:65]
        ones_view = bass.AP(tensor=vanchor.tensor, offset=vanchor.offset,
                            ap=[vanchor.ap[0], [256, 32], [1, 128]])
        nc.gpsimd.memset(ones_view.bitcast(f32), 1.0)

        # ---- initial DMAs, ordered by first consumption ----
        # wk column blocks (new order): 0:256 = kT01|q01, 256:512 = v01|kT23,
        # 512:768 = q23|v23
        xts0 = xpool.tile([128, 8, CHW], bf16, name="xts")
        for k in range(8):
            nc.sync.dma_start(xts0[:, k, :], xt_r[:, k, 0:CHW])
            nc.scalar.dma_start(wk[:, k, 0:256], wkqv_r[:, k, 0:256])
        nc.sync.dma_start(bkq_s, bkq)
        nc.sync.dma_start(ident, ident_d)
        nc.sync.dma_start(amask, amask_d.rearrange("p (a b) -> p a b", a=2))
        for k in range(8):
            (nc.sync if k % 2 else nc.scalar).dma_start(
                wk[:, k, 256:512], wkqv_r[:, k, 256:512])
        for k in range(8):
            (nc.sync if k % 2 else nc.scalar).dma_start(
                wk[:, k, 512:768], wkqv_r[:, k, 512:768])
        wp_r = wproj.rearrange("(kk p) f -> p kk f", p=128)
        nc.sync.dma_start(wp[:, 0, :], wp_r[:, 0, :])
        nc.scalar.dma_start(wp[:, 1, :], wp_r[:, 1, :])
        nc.sync.dma_start(bp_s, bp)

        xts_tiles = {0: xts0}

        def v_stationary(j, h):
            """[128,128] AP: even slot -> [v_A|ones64], odd -> [ones64|v_B]."""
            hf2, sl = h // 2, h % 2
            off = (j * 2 + hf2) * 256 + 128 * sl
            return vsb[:, off:off + 128]

        # ---- step emitters ----
        # m order: 0=kT01, 1=q01, 2=v01, 3=kT23, 4=q23, 5=v23
        M_DST = [(0, 0), (1, 0), (2, 0), (0, 1), (1, 1), (2, 1)]

        def qkv_m_step(n, m):
            with nc.named_scope(f"qkv{n}"):
                xts = xts_tiles[n]
                cols = slice(n * CHW, (n + 1) * CHW)
                acc = ps.tile([128, CHW], f32, name="acc", tag="acc")
                for k in range(8):
                    nc.tensor.matmul(acc, wk[:, k, m * 128:(m + 1) * 128],
                                     xts[:, k, :], start=(k == 0),
                                     stop=(k == 7))
                kind, hf = M_DST[m]
                dst = (ktp, qsb, vv)[kind]
                nc.vector.tensor_scalar_add(
                    out=dst[:, hf, cols], in0=acc, scalar1=bkq_s[:, m:m + 1])
                if m == 0 and n + 1 < NCH:
                    nxt = xpool.tile([128, 8, CHW], bf16, name="xts")
                    for k in range(8):
                        nc.sync.dma_start(
                            nxt[:, k, :],
                            xt_r[:, k, (n + 1) * CHW:(n + 2) * CHW])
                    xts_tiles[n + 1] = nxt

        def transpose_step(n, t2, hf):
            with nc.named_scope(f"attn{n}"):
                jb = 4 * n + 2 * t2
                tp = ps.tile([128, 2, 128], f32, name="tp", tag="acc")
                for ji in range(2):
                    nc.tensor.transpose(
                        tp[:, ji, :],
                        vv[:, hf,
                           (jb + ji) * 128:(jb + ji + 1) * 128].bitcast(f32),
                        ident)
                off = jb * 512 + hf * 256
                anch = vsb[:, off:off + 1]
                dst = bass.AP(tensor=anch.tensor, offset=anch.offset,
                              ap=[anch.ap[0], [512, 2], [192, 2], [1, 64]])
                nc.vector.tensor_copy(
                    dst, tp.rearrange("p j (a b) -> p j a b", a=2))

        pend = []

        def flush_pend(k=None):
            todo = len(pend) if k is None else k
            for _ in range(todo):
                pend.pop(0)()

        cur_pys = [None, None]

        def hf_start_step(n, hf):
            cur_pys[0] = ps.tile([128, CHW], f32, name="py0", tag="py")
            cur_pys[1] = ps.tile([128, CHW], f32, name="py1", tag="py")

        defer3 = []

        def qk_step(n, hf, j, defer=False):
            with nc.named_scope(f"attn{n}"):
                c0 = max(0, 128 * j - CHW * n)
                ss = ps.tile([128, 2, CHW], f32, name="ss", tag="ss")
                for sl in range(2):
                    nc.tensor.matmul(
                        ss[:, sl, c0:],
                        ktp[64 * sl:64 * sl + 64, hf, j * 128:(j + 1) * 128],
                        qsb[64 * sl:64 * sl + 64, hf,
                            n * CHW + c0:(n + 1) * CHW],
                        start=True, stop=True,
                        tile_position=(64 * sl, 0))
                if defer:
                    pt = ptp.tile([128, 2, CHW], f32r, name="pt3",
                                  tag="pt3", bufs=8)
                else:
                    pt = ptp.tile([128, 2, CHW], f32r, name="pt")
                nc.scalar.activation(out=pt[:, :, c0:], in_=ss[:, :, c0:],
                                     func=AF.Exp)
                if os.environ.get("KERNEL_DEBUG_DUMP") and (n, hf, j) == (1, 0, 0):
                    dbg3 = sing.tile([128, 2, CHW], f32, name="dbg3")
                    nc.vector.tensor_copy(dbg3[:, 0, :], ss[:, 0, :])
                    nc.vector.tensor_copy(dbg3[:, 1, :], pt[:, 0, :].bitcast(f32))
                    nc.sync.dma_start(dbgo[0:128, :], dbg3[:, 0, :])
                    nc.sync.dma_start(dbgo[128:256, :], dbg3[:, 1, :])
                if j >= 4 * n:  # diagonal block: 0/1 mask post-exp
                    nc.gpsimd.tensor_tensor(
                        out=pt[:, :, c0:c0 + 128],
                        in0=pt[:, :, c0:c0 + 128],
                        in1=amask, op=OP.mult)
                jmax = 4 * n + 3
                # deferred steps resolve the py accumulators at call time
                # (their hf_start runs later than this emission)
                pys = None if defer else tuple(cur_pys)

                def mk_pv(n=n, hf=hf, j=j, c0=c0, pt=pt, pys=pys, jmax=jmax):
                    py01 = tuple(cur_pys) if pys is None else pys
                    with nc.named_scope(f"attn{n}"):
                        for sl in range(2):
                            nc.tensor.matmul(
                                py01[sl][:, c0:],
                                v_stationary(j, 2 * hf + sl),
                                pt[:, sl, c0:],
                                start=(j == 0), stop=(j == jmax),
                                skip_group_check=True)
                if defer:
                    defer3.append(mk_pv)
                else:
                    pend.append(mk_pv)
                    if len(pend) > 2:
                        flush_pend(1)

        def norm_step(n, hf):
            with nc.named_scope(f"attn{n}"):
                flush_pend()
                pys = cur_pys
                # y of head A at 0:64 (pys0) / head B at 64:128 (pys1);
                # rowsum broadcasts live on the complements.  Copy rowsums
                # to SBUF, one full-partition fast reciprocal, DMA
                # partition-shift on the ACT queue, one gpsimd multiply.
                # The last norm routes its copies to the then-idle ACT so
                # the DVE chain shortens at the tail.
                tail = (n, hf) == (3, 1)

                def cpy(dst, src):
                    if tail:
                        nc.scalar.activation(out=dst, in_=src,
                                             func=AF.Identity, bias=0.0)
                    else:
                        nc.vector.tensor_copy(dst, src)

                pyc = rsp.tile([128, CHW], f32, name="pyc", tag="pyc")
                rcp = rsp.tile([128, CHW], f32, name="rcp", tag="rcp")
                rb = rbp.tile([128, CHW], f32, name="rb")
                if tail:
                    # column halves, sum-copies split ACT/DVE in parallel,
                    # so the first half of the reciprocals lands sooner
                    for ch in range(2):
                        s = slice(ch * 256, ch * 256 + 256)
                        nc.scalar.activation(out=pyc[0:64, s],
                                             in_=pys[1][0:64, s],
                                             func=AF.Identity, bias=0.0)
                        nc.vector.tensor_copy(pyc[64:128, s],
                                              pys[0][64:128, s])
                        nc.vector.reciprocal_approx_fast(out=rcp[:, s],
                                                         in_=pyc[:, s])
                        nc.scalar.dma_start(rb[0:64, s], rcp[64:128, s])
                        nc.scalar.dma_start(rb[64:128, s], rcp[0:64, s])
                else:
                    cpy(pyc[0:64, :], pys[1][0:64, :])
                    cpy(pyc[64:128, :], pys[0][64:128, :])
                    nc.vector.reciprocal_approx_fast(out=rcp, in_=pyc)
                    nc.scalar.dma_start(rb[0:64, :], rcp[64:128, :])
                    nc.scalar.dma_start(rb[64:128, :], rcp[0:64, :])
                pyy = rsp.tile([128, CHW], f32, name="pyy", tag="pyy")
                if tail:  # halves: first 256 cols feed proj3 kk=1 sooner
                    for ch in range(2):
                        cs = slice(ch * 256, ch * 256 + 256)
                        nc.vector.tensor_copy(pyy[0:64, cs],
                                              pys[0][0:64, cs])
                        nc.scalar.activation(out=pyy[64:128, cs],
                                             in_=pys[1][64:128, cs],
                                             func=AF.Identity, bias=0.0)
                        nc.gpsimd.tensor_tensor(
                            out=ysb[:, hf, n * CHW + ch * 256:
                                    n * CHW + ch * 256 + 256],
                            in0=pyy[:, cs], in1=rb[:, cs], op=OP.mult)
                else:
                    nc.vector.tensor_copy(pyy[0:64, :], pys[0][0:64, :])
                    nc.vector.tensor_copy(pyy[64:128, :], pys[1][64:128, :])
                    nc.gpsimd.tensor_tensor(
                        out=ysb[:, hf, n * CHW:(n + 1) * CHW],
                        in0=pyy, in1=rb, op=OP.mult)
                if os.environ.get("KERNEL_DEBUG_DUMP") and (n, hf) == (1, 0):
                    dbg2 = sing.tile([128, 4, CHW], f32, name="dbg2")
                    nc.vector.tensor_copy(dbg2[:, 0, :], pyy)
                    nc.vector.tensor_copy(dbg2[:, 1, :], pyc)
                    nc.vector.tensor_copy(dbg2[:, 2, :], rb)
                    nc.vector.tensor_copy(dbg2[:, 3, :], pys[0])
                    for q4 in range(4):
                        nc.sync.dma_start(
                            dbgo[256 + q4 * 128:256 + (q4 + 1) * 128, :],
                            dbg2[:, q4, :])

        p3_accs = []

        def p3a_step():
            # After chunk 3's last QK: flush the remaining PVs, then park
            # kk=0 partial projections for o=0..5 in the now-free ss/acc
            # PSUM slots.  These matmuls keep the PE warm while the final
            # normalization chain runs on ACT/DVE/DMA.
            flush_pend()
            with nc.named_scope("proj3"):
                ssA = ps.tile([128, 2, CHW], f32, name="p3sA", tag="ss")
                ssB = ps.tile([128, 2, CHW], f32, name="p3sB", tag="ss")
                aA = ps.tile([128, CHW], f32, name="p3aA", tag="acc")
                aB = ps.tile([128, CHW], f32, name="p3aB", tag="acc")
                p3_accs.extend([ssA[:, 0, :], ssA[:, 1, :],
                                ssB[:, 0, :], ssB[:, 1, :], aA, aB])
                for o in range(6):
                    nc.tensor.matmul(
                        p3_accs[o], wp[:, 0, o * 128:(o + 1) * 128],
                        ysb[:, 0, 3 * CHW:4 * CHW],
                        start=True, stop=False, skip_group_check=True)

        def p3b_epilogue():
            with nc.named_scope("proj3"):
                for o in range(6, 8):
                    acc = ps.tile([128, CHW], f32, name="p3p", tag="py")
                    p3_accs.append(acc)
                    nc.tensor.matmul(
                        acc, wp[:, 0, o * 128:(o + 1) * 128],
                        ysb[:, 0, 3 * CHW:4 * CHW],
                        start=True, stop=False, skip_group_check=True)
                # kk=1 in column halves; once kk0(full) + kk1-h0 land, the
                # first 256 columns of an accumulator are final -- copy and
                # DMA them out while the second half still computes
                for hh in range(2):
                    cs = slice(3 * CHW + hh * 256, 3 * CHW + hh * 256 + 256)
                    for o in range(8):
                        nc.tensor.matmul(
                            p3_accs[o][:, hh * 256:hh * 256 + 256],
                            wp[:, 1, o * 128:(o + 1) * 128],
                            ysb[:, 1, cs],
                            start=False, stop=(hh == 1),
                            skip_group_check=True)
                        ot = osp.tile([128, 256], bf16, name="ot")
                        src = p3_accs[o][:, hh * 256:hh * 256 + 256]
                        if o % 2 == 0:
                            nc.scalar.activation(out=ot, in_=src,
                                                 func=AF.Identity,
                                                 bias=bp_s[:, o:o + 1])
                        else:
                            nc.vector.tensor_scalar_add(
                                out=ot, in0=src, scalar1=bp_s[:, o:o + 1])
                        eng = nc.sync if o % 2 == 0 else nc.scalar
                        eng.dma_start(
                            outp[o * 128:(o + 1) * 128, cs], ot)

        def proj_step(n, o):
            with nc.named_scope(f"proj{n}"):
                tag = "acc" if (n < 3 or o % 2 == 0) else "ss"
                acc = ps.tile([128, CHW], f32, name="pacc", tag=tag)
                for kk in range(2):
                    nc.tensor.matmul(acc, wp[:, kk, o * 128:(o + 1) * 128],
                                     ysb[:, kk, n * CHW:(n + 1) * CHW],
                                     start=(kk == 0), stop=(kk == 1))
                ot = osp.tile([128, CHW], bf16, name="ot")
                if n < 3:
                    nc.vector.tensor_scalar_add(out=ot, in0=acc,
                                                scalar1=bp_s[:, o:o + 1])
                else:
                    nc.scalar.activation(out=ot, in_=acc, func=AF.Identity,
                                         bias=bp_s[:, o:o + 1])
                nc.sync.dma_start(
                    outp[o * 128:(o + 1) * 128,
                         n * CHW:n * CHW + 256], ot[:, 0:256])
                nc.sync.dma_start(
                    outp[o * 128:(o + 1) * 128,
                         n * CHW + 256:(n + 1) * CHW], ot[:, 256:512])

        def dispatch(step):
            kind = step[0]
            if kind == "qk":
                qk_step(*step[1:])
            elif kind == "qkvm":
                qkv_m_step(*step[1:])
            elif kind == "tr":
                transpose_step(*step[1:])
            elif kind == "hfs":
                hf_start_step(*step[1:])
            elif kind == "norm":
                norm_step(*step[1:])
            elif kind == "proj":
                proj_step(*step[1:])
            elif kind == "p3a":
                p3a_step()
            elif kind == "qk3e":
                qk_step(3, 0, step[1], defer=True)
            elif kind == "fl3":
                while defer3:
                    defer3.pop(0)()
            else:
                raise AssertionError(kind)

        # ---- prologue: PE warmup on garbage inputs + first head-pair's
        # QKV (m0 = kT01, m1 = q01) ----
        nc.vector.memset(warm, 1.0)
        # preload the exp table set on ACT while the first DMAs land
        wexp = sing.tile([128, 8], f32, name="wexp")
        nc.scalar.activation(out=wexp, in_=warm[:, 0:8], func=AF.Exp)
        for w in range(14):
            wps = ps.tile([128, CHW], f32, name="wps", tag="acc")
            nc.tensor.matmul(wps, warm[:, 0:128], warm, start=True, stop=True)
        qkv_m_step(0, 0)
        qkv_m_step(0, 1)

        # chunk 0 schedule, hand-ordered (v01 -> transposes(hf0) before the
        # first PV flush; kT23/q23 before hf1's QKs)
        sched = [("hfs", 0, 0), ("qk", 0, 0, 0), ("qkvm", 0, 2),
                 ("qk", 0, 0, 1), ("tr", 0, 0, 0), ("tr", 0, 1, 0),
                 ("qk", 0, 0, 2), ("qkvm", 0, 3), ("qk", 0, 0, 3),
                 ("qkvm", 0, 4), ("norm", 0, 0),
                 ("hfs", 0, 1), ("qk", 0, 1, 0), ("qkvm", 0, 5),
                 ("qk", 0, 1, 1), ("tr", 0, 0, 1), ("tr", 0, 1, 1),
                 ("qk", 0, 1, 2), ("qkvm", 1, 0), ("qk", 0, 1, 3),
                 ("qkvm", 1, 1), ("norm", 0, 1),
                 ("qkvm", 1, 2), ("qkvm", 1, 3), ("qkvm", 1, 4),
                 ("qkvm", 1, 5)]
        for s in sched:
            dispatch(s)

        # chunks 1..3: attention interleaved with next chunk's QKV and
        # previous chunk's projection
        for n in range(1, NCH):
            attn = []
            for hf in range(2):
                attn.append(("hfs", n, hf))
                if n == 3 and hf == 0:
                    # j0..7's QK+exp ran during chunk 2; their PVs flush here
                    attn.append(("fl3",))
                j0 = 8 if (n == 3 and hf == 0) else 0
                for j in range(j0, 4 * n + 4):
                    attn.append(("qk", n, hf, j))
                attn.append(("norm", n, hf))
            # v transposes for this chunk, per head-pair, early in each hf
            anchor1 = ("qk", n, 0, 9) if n == 3 else ("qk", n, 0, 1)
            base1 = attn.index(anchor1) + 1
            attn.insert(base1, ("tr", n, 0, 0))
            attn.insert(base1 + 1, ("tr", n, 1, 0))
            base2 = attn.index(("qk", n, 1, 1)) + 1
            attn.insert(base2, ("tr", n, 0, 1))
            attn.insert(base2 + 1, ("tr", n, 1, 1))
            if n == 3:
                attn.insert(attn.index(("norm", 3, 1)), ("p3a",))

            # qkvm fillers pace over the first ~2/3 of qk steps; proj
            # fillers start a few steps in (the previous chunk's last norm
            # chain is still draining) and pace over the rest.  Chunk 3 is
            # ACT(exp)-limited, so both proj(1) and proj(2) land there as
            # extra PE work.
            qkvm_f = ([("qkvm", n + 1, m) for m in range(6)]
                      if n + 1 < NCH else [])
            if n == 1:
                proj_f = [("proj", 0, o) for o in range(8)]
            elif n == 2:
                proj_f = [("proj", 1, o) for o in range(4)]
            else:
                proj_f = [("proj", 1, o) for o in range(4, 8)]
                proj_f += [("proj", 2, o) for o in range(8)]
            # chunk 2 also hosts chunk 3 / hf0's early QK+exp steps
            e3 = [("qk3e", j) for j in range(8)] if n == 2 else []
            nqk = sum(1 for s in attn if s[0] == "qk")
            span_a = max(1, (2 * nqk) // 3)
            qi = ai = pi = ei = 0
            for s in attn:
                dispatch(s)
                if s[0] == "qk":
                    qi += 1
                    want_a = min(len(qkvm_f), (qi * len(qkvm_f)) // span_a)
                    while ai < want_a:
                        dispatch(qkvm_f[ai])
                        ai += 1
                    # complete proj fillers by ~5/8 of the chunk so their
                    # output DMAs drain before the tail
                    span_p = max(1, (5 * nqk) // 8 - 3)
                    if qi > 3:
                        want_p = min(len(proj_f),
                                     ((qi - 3) * len(proj_f)) // span_p)
                        while pi < want_p:
                            dispatch(proj_f[pi])
                            pi += 1
                    if e3 and qi > 8:
                        want_e = min(len(e3),
                                     ((qi - 8) * len(e3)) // (nqk - 8))
                        while ei < want_e:
                            dispatch(e3[ei])
                            ei += 1
            while ai < len(qkvm_f):
                dispatch(qkvm_f[ai])
                ai += 1
            while pi < len(proj_f):
                dispatch(proj_f[pi])
                pi += 1
            while ei < len(e3):
                dispatch(e3[ei])
                ei += 1

        # epilogue: finish the last chunk's projection (kk=1 accumulate +
        # bias copies on ACT + output DMA)
        p3b_epilogue()

    nc.compile()
    return nc


def _host_inputs(x, W_kqv, b_kqv, W_proj, b_proj):
    import ml_dtypes

    bf16 = ml_dtypes.bfloat16
    x = np.ascontiguousarray(np.asarray(x, dtype=np.float32))
    W_kqv = np.asarray(W_kqv, dtype=np.float32)
    b_kqv = np.asarray(b_kqv, dtype=np.float32)
    W_proj = np.asarray(W_proj, dtype=np.float32)
    b_proj = np.asarray(b_proj, dtype=np.float32)

    ident = np.eye(128, dtype=np.float32)
    ss, tt = np.meshgrid(np.arange(128), np.arange(128), indexing="ij")
    amask = np.tile((ss <= tt).astype(np.float32), (1, 2))  # 0/1 mult mask

    xts = [np.ascontiguousarray(x[b].T.astype(bf16)) for b in range(B)]

    in_maps = []
    for c in range(NCORES):
        b, g = c // 4, c % 4
        heads = [4 * g + i for i in range(HPC)]

        def kqv_block(pair):
            hs = heads[2 * pair:2 * pair + 2]
            return [np.concatenate([W_kqv[h * 192:h * 192 + 64]
                                    for h in hs], axis=0),
                    np.concatenate([W_kqv[h * 192 + 64:h * 192 + 128] * 0.125
                                    for h in hs], axis=0),
                    np.concatenate([W_kqv[h * 192 + 128:h * 192 + 192]
                                    for h in hs], axis=0)]

        def bias_block(pair):
            hs = heads[2 * pair:2 * pair + 2]
            return [np.concatenate([b_kqv[h * 192:h * 192 + 64]
                                    for h in hs]),
                    np.concatenate([b_kqv[h * 192 + 64:h * 192 + 128] * 0.125
                                    for h in hs]),
                    np.concatenate([b_kqv[h * 192 + 128:h * 192 + 192]
                                    for h in hs])]

        # m order: kT01, q01, v01, kT23, q23, v23
        wl = np.concatenate(kqv_block(0) + kqv_block(1), axis=0)
        bl = np.concatenate(bias_block(0) + bias_block(1))
        bpl = b_proj if g == 0 else np.zeros_like(b_proj)
        in_maps.append({
            "xt": xts[b],
            "wkqv": np.ascontiguousarray(wl.T.astype(bf16)),
            "bkq": np.ascontiguousarray(bl.reshape(6, 128).T),
            "bp": np.ascontiguousarray(bpl.reshape(8, 128).T),
            "wproj": np.ascontiguousarray(
                W_proj[:, 256 * g:256 * (g + 1)].T.astype(bf16)),
            "ident": ident,
            "amask": amask,
        })
    return in_maps


def kernel(x, W_kqv, b_kqv, W_proj, b_proj):
    from concourse.bass_utils import run_bass_kernel_spmd

    if "nc" not in _cache:
        _cache["nc"] = _build_nc()
    nc = _cache["nc"]

    in_maps = _host_inputs(x, W_kqv, b_kqv, W_proj, b_proj)
    trace = bool(int(os.environ.get("KERNEL_TRACE", "0")))
    r = run_bass_kernel_spmd(nc, in_maps, core_ids=list(range(NCORES)),
                             trace=trace)
    if trace:
        _cache["last_results"] = r
        print(f"HW exec time: {r.exec_time_ns} ns")

    out = np.empty((B, T, C), dtype=np.float32)
    for b in range(B):
        acc = np.zeros((C, T), dtype=np.float32)
        for g in range(4):
            acc += r.results[4 * b + g]["outp"].astype(np.float32)
        out[b] = acc.T
    return out
